# revision 2
# baseline (speedup 1.0000x reference)
"""Trainium2 Bass kernel for nn_Attention_3728031613575.

Multi-head attention, B=4 L=2048 D=1024 H=16 (head dim 64), fp32 reference:
    q/k/v = split_heads(x @ W{q,k,v} + b)        [b,h,l,64]
    scores = q k^T + mask * (-1e5)
    out    = softmax(scores) @ v                 -> [b,l,1024]

Sharding (8 cores): core c handles batch b = c//2 and heads (c%2)*8..+8
(batch x head-group data parallel; QKV weights column-sharded by head).
Attention is fully local per core; no collectives.

Per-core algorithm (all layouts chosen to keep softmax on the PSUM
partition dim without any probability transposes):
  - X^T built once via PE transposes (fp32r).
  - Q^T/K^T [head-dims, l] and V [l, head-dims] projections in fp32r,
    biases folded in as rank-1 matmul terms.
  - mask preprocessed once per core:  M_e = exp(-1e5*(m - rowmin(m)))
    (the rowmin bias makes the softmax max-subtraction exact for the
    mask-dominated term; the remaining q.k part is range-safe in fp32),
    stored bf16, reloaded transposed via the DMA xbar.
  - per (head-pair, q-superblock 1024, k-block 128):
      S^T[k,q] = K^T.T @ Q^T   (two K=64 matmuls row-tiled on the PE)
      E = exp(S^T) (ACT, bf16)
      P~ = E * M_e^T (DVE, bf16)
      O'^T[d,q] += V^T P~ with a ones-column in V producing the softmax
      denominators as row 64 of O'.
  - postproc: PE-transpose O'^T -> [q, 65], reciprocal of col 64,
    tensor_scalar normalize, DMA out.
"""

import sys

sys.path.insert(0, "/opt/trn_rl_repo")

import numpy as np

B, L, D, H, DH = 4, 2048, 1024, 16, 64
NCORES = 8
HPC = 8            # heads per core
NPAIR = HPC // 2   # head pairs per core
NSB = 2            # q superblocks per core
SBW = L // NSB     # 1024
NKB = L // 128     # 16 k-blocks
NDB = D // 128     # 8 contraction chunks
MASK_C = -100000.0

_CACHE = {}


def _build():
    """Build and finalize the per-core Bass module (same program, SPMD)."""
    import concourse.bass as bass
    from concourse import bacc, mybir
    import concourse.tile as tile
    from concourse.masks import make_identity

    F32 = mybir.dt.float32
    F32R = mybir.dt.float32r
    BF16 = mybir.dt.bfloat16
    AF = mybir.ActivationFunctionType
    ALU = mybir.AluOpType
    AX = mybir.AxisListType

    nc = bacc.Bacc(None, target_bir_lowering=False)

    x_d = nc.dram_tensor("x", [L, D], F32R, kind="ExternalInput")
    mask_d = nc.dram_tensor("mask", [L, L], F32, kind="ExternalInput")
    wq_d = nc.dram_tensor("wq", [D, 512], F32R, kind="ExternalInput")
    wk_d = nc.dram_tensor("wk", [D, 512], F32R, kind="ExternalInput")
    wv_d = nc.dram_tensor("wv", [D, 512], F32R, kind="ExternalInput")
    bq_d = nc.dram_tensor("bq", [1, 512], F32R, kind="ExternalInput")
    bk_d = nc.dram_tensor("bk", [1, 512], F32R, kind="ExternalInput")
    bv_d = nc.dram_tensor("bv", [1, 512], F32R, kind="ExternalInput")
    out_d = nc.dram_tensor("out", [L, 512], F32, kind="ExternalOutput")

    with tile.TileContext(nc) as tc:
        with tc.tile_pool(name="const", bufs=1) as constp, \
             tc.tile_pool(name="persist", bufs=1) as pers, \
             tc.tile_pool(name="dram", bufs=1, space="DRAM") as dramp:

            # ---- constants
            idf32 = constp.tile([128, 128], F32, name="idf32", tag="idf32")
            make_identity(nc, idf32)
            idf32r = constp.tile([128, 128], F32R, name="idf32r", tag="idf32r")
            nc.vector.tensor_copy(idf32r, idf32)
            ones_f = constp.tile([1, 512], F32, name="ones_f", tag="ones_f")
            nc.vector.memset(ones_f, 1.0)
            ones_r = constp.tile([1, 512], F32R, name="ones_r", tag="ones_r")
            nc.vector.tensor_copy(ones_r, ones_f)

            # ---- persistent activations
            QT = pers.tile([128, NPAIR, L], F32R, name="QT", tag="QT")
            KT = pers.tile([128, NPAIR, L], F32R, name="KT", tag="KT")
            V = pers.tile([128, NKB, HPC, DH + 1], BF16, name="V", tag="V")
            nc.vector.memset(V[:, :, :, DH], 1.0)

            me_dram = dramp.tile([L, L], BF16, name="me_dram", tag="me_dram")

            # ================= phase 1: X^T + QKV projections =================
            with tc.tile_pool(name="wpool", bufs=1) as wpool, \
                 tc.tile_pool(name="xload", bufs=2) as xload, \
                 tc.tile_pool(name="xtpool", bufs=2) as xtpool, \
                 tc.tile_pool(name="tpsum", bufs=2, space="PSUM") as tpsum, \
                 tc.tile_pool(name="qkvpsum", bufs=4, space="PSUM") as qkvpsum:

                wq = wpool.tile([128, NDB, 512], F32R, name="wq", tag="wq")
                wk = wpool.tile([128, NDB, 512], F32R, name="wk", tag="wk")
                wv = wpool.tile([128, NDB, 512], F32R, name="wv", tag="wv")
                nc.sync.dma_start(out=wq, in_=wq_d.rearrange("(c p) n -> p c n", p=128))
                nc.sync.dma_start(out=wk, in_=wk_d.rearrange("(c p) n -> p c n", p=128))
                nc.sync.dma_start(out=wv, in_=wv_d.rearrange("(c p) n -> p c n", p=128))
                bq = wpool.tile([1, 512], F32R, name="bq", tag="bq")
                bk = wpool.tile([1, 512], F32R, name="bk", tag="bk")
                bv = wpool.tile([1, 512], F32R, name="bv", tag="bv")
                nc.sync.dma_start(out=bq, in_=bq_d[:, :])
                nc.sync.dma_start(out=bk, in_=bk_d[:, :])
                nc.sync.dma_start(out=bv, in_=bv_d[:, :])

                for lb in range(4):            # l superblocks of 512
                    xl = xload.tile([128, 4, D], F32R, name=f"xl{lb}", tag="xl")
                    nc.sync.dma_start(
                        out=xl,
                        in_=x_d[lb * 512:(lb + 1) * 512, :].rearrange(
                            "(s p) d -> p s d", p=128),
                    )
                    xt = xtpool.tile([128, NDB, 512], F32R, name=f"xt{lb}", tag="xt")
                    for db in range(NDB):
                        tp = tpsum.tile([128, 512], F32R, name=f"tp{lb}_{db}", tag="tp")
                        for s in range(4):
                            nc.tensor.transpose(
                                tp[:, s * 128:(s + 1) * 128],
                                xl[:, s, db * 128:(db + 1) * 128],
                                idf32r,
                            )
                        nc.vector.tensor_copy(xt[:, db, :], tp)

                    # Q^T / K^T blocks: out [128 head-dims, 512 l]
                    for w_sb, b_sb, dst in ((wq, bq, QT), (wk, bk, KT)):
                        for np_ in range(NPAIR):
                            qp = qkvpsum.tile([128, 512], F32,
                                              name=f"qp{lb}_{np_}", tag="qp")
                            for db in range(NDB):
                                nc.tensor.matmul(
                                    qp,
                                    w_sb[:, db, np_ * 128:(np_ + 1) * 128],
                                    xt[:, db, :],
                                    start=(db == 0), stop=False)
                            nc.tensor.matmul(
                                qp, b_sb[0:1, np_ * 128:(np_ + 1) * 128],
                                ones_r, start=False, stop=True)
                            nc.vector.tensor_copy(
                                dst[:, np_, lb * 512:(lb + 1) * 512], qp)

                    # V blocks: out [128 l, 512 head-dims]
                    for s in range(4):
                        kb = lb * 4 + s
                        vp = qkvpsum.tile([128, 512], F32, name=f"vp{kb}", tag="qp")
                        for db in range(NDB):
                            nc.tensor.matmul(
                                vp,
                                xt[:, db, s * 128:(s + 1) * 128],
                                wv[:, db, :],
                                start=(db == 0), stop=False)
                        nc.tensor.matmul(vp, ones_r[0:1, 0:128], bv,
                                         start=False, stop=True)
                        nc.vector.tensor_copy(
                            V[:, kb, :, 0:DH],
                            vp.rearrange("p (h d) -> p h d", h=HPC))

            # ================= phase 2: mask -> M_e (bf16, DRAM) =============
            with tc.tile_pool(name="mload", bufs=3) as mload, \
                 tc.tile_pool(name="mtmp", bufs=3) as mtmp:
                for qb in range(16):
                    ml = mload.tile([128, L], F32, name=f"ml{qb}", tag="ml")
                    nc.sync.dma_start(out=ml,
                                      in_=mask_d[qb * 128:(qb + 1) * 128, :])
                    mmin = mtmp.tile([128, 1], F32, name=f"mmin{qb}", tag="mmin")
                    nc.vector.tensor_reduce(mmin, ml, axis=AX.X, op=ALU.min)
                    mbias = mtmp.tile([128, 1], F32, name=f"mb{qb}", tag="mb")
                    nc.vector.tensor_scalar_mul(mbias, mmin, -MASK_C)
                    me = mtmp.tile([128, L], BF16, name=f"me{qb}", tag="me")
                    nc.scalar.activation(me, ml, AF.Exp, bias=mbias, scale=MASK_C)
                    nc.sync.dma_start(out=me_dram[qb * 128:(qb + 1) * 128, :],
                                      in_=me)

            # ================= phase 3: attention =============================
            with tc.tile_pool(name="met", bufs=1) as metp, \
                 tc.tile_pool(name="stage", bufs=2) as stagep, \
                 tc.tile_pool(name="epool", bufs=2) as epool, \
                 tc.tile_pool(name="ppool", bufs=3) as ppool, \
                 tc.tile_pool(name="oevac", bufs=2) as oevacp, \
                 tc.tile_pool(name="rpool", bufs=4) as rpool, \
                 tc.tile_pool(name="spsum", bufs=1, space="PSUM") as spsum, \
                 tc.tile_pool(name="opsum", bufs=2, space="PSUM") as opsum:

                for sb in range(NSB):
                    met = metp.tile([128, NKB, SBW], BF16,
                                    name=f"met{sb}", tag="met")
                    for kb in range(NKB):
                        nc.sync.dma_start_transpose(
                            met[:, kb, :],
                            me_dram[sb * SBW:(sb + 1) * SBW,
                                    kb * 128:(kb + 1) * 128])
                    stage = stagep.tile([128, 8, 512], F32,
                                        name=f"st{sb}", tag="st")

                    for pr in range(NPAIR):
                        hA, hB = 2 * pr, 2 * pr + 1
                        oa = opsum.tile([DH + 1, SBW], F32,
                                        name=f"oa{sb}_{pr}", tag="o")
                        ob = opsum.tile([DH + 1, SBW], F32,
                                        name=f"ob{sb}_{pr}", tag="o")
                        for kb in range(NKB):
                            sp = spsum.tile([128, 2048], F32,
                                            name=f"sp{sb}_{pr}_{kb}", tag="s")
                            for half, rs, tpos in ((0, slice(0, 64), (0, 0)),
                                                   (1, slice(64, 128), (64, 0))):
                                for qh in range(2):
                                    nc.tensor.matmul(
                                        sp[:, half * 1024 + qh * 512:
                                           half * 1024 + qh * 512 + 512],
                                        KT[rs, pr, kb * 128:(kb + 1) * 128],
                                        QT[rs, pr, sb * SBW + qh * 512:
                                           sb * SBW + qh * 512 + 512],
                                        start=True, stop=True,
                                        tile_position=tpos)
                            ep = epool.tile([128, 2048], BF16,
                                            name=f"e{sb}_{pr}_{kb}", tag="e")
                            nc.scalar.activation(ep, sp, AF.Exp)
                            pp = ppool.tile([128, 2048], BF16,
                                            name=f"pp{sb}_{pr}_{kb}", tag="pp")
                            mdup = bass.AP(
                                tensor=met.tensor,
                                offset=met[:, kb, :].offset,
                                ap=[met.ap[0], [0, 2], [1, SBW]])
                            nc.vector.tensor_tensor(
                                out=pp.rearrange("p (r f) -> p r f", r=2),
                                in0=ep.rearrange("p (r f) -> p r f", r=2),
                                in1=mdup, op=ALU.mult)
                            for half, o_ps, h in ((0, oa, hA), (1, ob, hB)):
                                for qh in range(2):
                                    nc.tensor.matmul(
                                        o_ps[:, qh * 512:(qh + 1) * 512],
                                        V[:, kb, h, :],
                                        pp[:, half * 1024 + qh * 512:
                                           half * 1024 + qh * 512 + 512],
                                        start=(kb == 0), stop=(kb == NKB - 1))

                        # ---- postproc both heads of the pair
                        osbA = oevacp.tile([DH + 1, SBW], F32,
                                           name=f"oeA{sb}_{pr}", tag="oe")
                        osbB = oevacp.tile([DH + 1, SBW], F32,
                                           name=f"oeB{sb}_{pr}", tag="oe")
                        nc.vector.tensor_copy(osbA, oa)
                        nc.vector.tensor_copy(osbB, ob)
                        for osb, h in ((osbA, hA), (osbB, hB)):
                            hcol = h * DH
                            for g in range(2):
                                tp = opsum.tile([128, 4 * 65], F32,
                                                name=f"tp{sb}_{pr}_{h}_{g}",
                                                tag="o")
                                for j in range(4):
                                    qb = g * 4 + j
                                    nc.tensor.transpose(
                                        tp[:, j * 65:(j + 1) * 65],
                                        osb[:, qb * 128:(qb + 1) * 128],
                                        idf32[0:65, 0:65])
                                tpv = tp.rearrange("p (j c) -> p j c", j=4)
                                rec = rpool.tile([128, 4], F32,
                                                 name=f"rc{sb}_{pr}_{h}_{g}",
                                                 tag="rc")
                                nc.vector.reciprocal(rec, tpv[:, :, 64:65])
                                for j in range(4):
                                    nc.vector.tensor_scalar_mul(
                                        stage[:, g * 4 + j, hcol:hcol + DH],
                                        tpv[:, j, 0:DH],
                                        rec[:, j:j + 1])

                    for qb in range(8):
                        nc.sync.dma_start(
                            out=out_d[sb * SBW + qb * 128:
                                      sb * SBW + (qb + 1) * 128, :],
                            in_=stage[:, qb, :])

    nc.finalize()
    return nc


def _get_nc():
    if "nc" not in _CACHE:
        _CACHE["nc"] = _build()
    return _CACHE["nc"]


def kernel(embedding, mask, Wq, bq, Wk, bk, Wv, bv):
    from concourse.bass_utils import run_bass_kernel_spmd

    nc = _get_nc()

    embedding = np.asarray(embedding, dtype=np.float32)
    mask = np.asarray(mask, dtype=np.float32)
    in_maps = []
    for c in range(NCORES):
        b = c // 2
        h0 = (c % 2) * HPC
        cs = slice(h0 * DH, (h0 + HPC) * DH)
        in_maps.append({
            "x": np.ascontiguousarray(embedding[b]),
            "mask": np.ascontiguousarray(mask[b, 0]),
            "wq": np.ascontiguousarray(np.asarray(Wq, np.float32)[:, cs]),
            "wk": np.ascontiguousarray(np.asarray(Wk, np.float32)[:, cs]),
            "wv": np.ascontiguousarray(np.asarray(Wv, np.float32)[:, cs]),
            "bq": np.ascontiguousarray(np.asarray(bq, np.float32)[cs]).reshape(1, 512),
            "bk": np.ascontiguousarray(np.asarray(bk, np.float32)[cs]).reshape(1, 512),
            "bv": np.ascontiguousarray(np.asarray(bv, np.float32)[cs]).reshape(1, 512),
        })

    res = run_bass_kernel_spmd(nc, in_maps, core_ids=list(range(NCORES)))

    out = np.empty((B, L, D), dtype=np.float32)
    for c in range(NCORES):
        b = c // 2
        h0 = (c % 2) * HPC
        out[b][:, h0 * DH:(h0 + HPC) * DH] = res.results[c]["out"]
    return out


# revision 6
# speedup vs baseline: 1.0119x; 1.0119x over previous
"""Trainium2 Bass kernel for nn_Attention_3728031613575.

Multi-head attention, B=4 L=2048 D=1024 H=16 (head dim 64), fp32 reference:
    q/k/v = split_heads(x @ W{q,k,v} + b)        [b,h,l,64]
    scores = q k^T + mask * (-1e5)
    out    = softmax(scores) @ v                 -> [b,l,1024]

Sharding (8 cores): core c handles batch b = c//2 and heads (c%2)*8..+8
(batch x head-group data parallel; QKV weights column-sharded by head).
Attention is fully local per core; no collectives.

Per-core algorithm (all layouts chosen to keep softmax on the PSUM
partition dim without any probability transposes):
  - X^T built once via PE transposes (fp32r).
  - Q^T/K^T [head-dims, l] and V [l, head-dims] projections in fp32r,
    biases folded in as rank-1 matmul terms.
  - mask preprocessed once per core:  M_e = exp(-1e5*(m - rowmin(m)))
    (the rowmin bias makes the softmax max-subtraction exact for the
    mask-dominated term; the remaining q.k part is range-safe in fp32),
    stored bf16, reloaded transposed via the DMA xbar.
  - per (head-pair, q-superblock 1024, k-block 128):
      S^T[k,q] = K^T.T @ Q^T   (two K=64 matmuls row-tiled on the PE)
      E = exp(S^T) (ACT, bf16)
      P~ = E * M_e^T (DVE, bf16)
      O'^T[d,q] += V^T P~ with a ones-column in V producing the softmax
      denominators as row 64 of O'.
  - postproc: PE-transpose O'^T -> [q, 65], reciprocal of col 64,
    tensor_scalar normalize, DMA out.
"""

import sys

sys.path.insert(0, "/opt/trn_rl_repo")

import numpy as np

B, L, D, H, DH = 4, 2048, 1024, 16, 64
NCORES = 8
HPC = 8            # heads per core
NPAIR = HPC // 2   # head pairs per core
NSB = 2            # q superblocks per core
SBW = L // NSB     # 1024
NKB = L // 128     # 16 k-blocks
NDB = D // 128     # 8 contraction chunks
MASK_C = -100000.0

_CACHE = {}


def _build():
    """Build and finalize the per-core Bass module (same program, SPMD)."""
    import concourse.bass as bass
    from concourse import bacc, mybir
    import concourse.tile as tile
    from concourse.masks import make_identity

    F32 = mybir.dt.float32
    F32R = mybir.dt.float32r
    BF16 = mybir.dt.bfloat16
    AF = mybir.ActivationFunctionType
    ALU = mybir.AluOpType
    AX = mybir.AxisListType

    nc = bacc.Bacc(None, target_bir_lowering=False)

    x_d = nc.dram_tensor("x", [L, D], F32R, kind="ExternalInput")
    mask_d = nc.dram_tensor("mask", [L, L], F32, kind="ExternalInput")
    wq_d = nc.dram_tensor("wq", [D, 512], F32R, kind="ExternalInput")
    wk_d = nc.dram_tensor("wk", [D, 512], F32R, kind="ExternalInput")
    wv_d = nc.dram_tensor("wv", [D, 512], F32R, kind="ExternalInput")
    bq_d = nc.dram_tensor("bq", [1, 512], F32R, kind="ExternalInput")
    bk_d = nc.dram_tensor("bk", [1, 512], F32R, kind="ExternalInput")
    bv_d = nc.dram_tensor("bv", [1, 512], F32R, kind="ExternalInput")
    out_d = nc.dram_tensor("out", [L, 512], F32, kind="ExternalOutput")

    with tile.TileContext(nc) as tc:
        with tc.tile_pool(name="const", bufs=1) as constp, \
             tc.tile_pool(name="persist", bufs=1) as pers, \
             tc.tile_pool(name="dram", bufs=1, space="DRAM") as dramp:

            # ---- constants
            idf32 = constp.tile([128, 128], F32, name="idf32", tag="idf32")
            make_identity(nc, idf32)
            idf32r = constp.tile([128, 128], F32R, name="idf32r", tag="idf32r")
            nc.vector.tensor_copy(idf32r, idf32)
            ones_f = constp.tile([1, 512], F32, name="ones_f", tag="ones_f")
            nc.vector.memset(ones_f, 1.0)
            ones_r = constp.tile([1, 512], F32R, name="ones_r", tag="ones_r")
            nc.vector.tensor_copy(ones_r, ones_f)

            # ---- persistent activations
            QT = pers.tile([128, NPAIR, L], F32R, name="QT", tag="QT")
            KT = pers.tile([128, NPAIR, L], F32R, name="KT", tag="KT")
            V = pers.tile([128, NKB, HPC, DH + 1], BF16, name="V", tag="V")
            nc.vector.memset(V[:, :, :, DH], 1.0)

            me_dram = dramp.tile([L, L], BF16, name="me_dram", tag="me_dram")

            # ================= phase 2: mask -> M_e (bf16, DRAM) =============
            # Emitted first: its ACT/DVE/DMA work overlaps the PE-heavy
            # projection phase below.
            with tc.tile_pool(name="mload", bufs=3) as mload, \
                 tc.tile_pool(name="mtmp", bufs=3) as mtmp:
                for qb in range(16):
                    ml = mload.tile([128, L], F32, name=f"ml{qb}", tag="ml")
                    nc.sync.dma_start(out=ml,
                                      in_=mask_d[qb * 128:(qb + 1) * 128, :])
                    mmin = mtmp.tile([128, 1], F32, name=f"mmin{qb}", tag="mmin")
                    nc.vector.tensor_reduce(mmin, ml, axis=AX.X, op=ALU.min)
                    mbias = mtmp.tile([128, 1], F32, name=f"mb{qb}", tag="mb")
                    nc.vector.tensor_scalar_mul(mbias, mmin, -MASK_C)
                    me = mtmp.tile([128, L], BF16, name=f"me{qb}", tag="me")
                    nc.scalar.activation(me, ml, AF.Exp, bias=mbias, scale=MASK_C)
                    nc.sync.dma_start(out=me_dram[qb * 128:(qb + 1) * 128, :],
                                      in_=me)

            # ================= phase 1: X^T + QKV projections =================
            with tc.tile_pool(name="wpool", bufs=1) as wpool, \
                 tc.tile_pool(name="xload", bufs=2) as xload, \
                 tc.tile_pool(name="xtpool", bufs=2) as xtpool, \
                 tc.tile_pool(name="tpsum", bufs=2, space="PSUM") as tpsum, \
                 tc.tile_pool(name="qkvpsum", bufs=4, space="PSUM") as qkvpsum:

                wq = wpool.tile([128, NDB, 512], F32R, name="wq", tag="wq")
                wk = wpool.tile([128, NDB, 512], F32R, name="wk", tag="wk")
                wv = wpool.tile([128, NDB, 512], F32R, name="wv", tag="wv")
                nc.sync.dma_start(out=wq, in_=wq_d.rearrange("(c p) n -> p c n", p=128))
                nc.sync.dma_start(out=wk, in_=wk_d.rearrange("(c p) n -> p c n", p=128))
                nc.sync.dma_start(out=wv, in_=wv_d.rearrange("(c p) n -> p c n", p=128))
                bq = wpool.tile([1, 512], F32R, name="bq", tag="bq")
                bk = wpool.tile([1, 512], F32R, name="bk", tag="bk")
                bv = wpool.tile([1, 512], F32R, name="bv", tag="bv")
                nc.sync.dma_start(out=bq, in_=bq_d[:, :])
                nc.sync.dma_start(out=bk, in_=bk_d[:, :])
                nc.sync.dma_start(out=bv, in_=bv_d[:, :])

                for lb in range(4):            # l superblocks of 512
                    xl = xload.tile([128, 4, D], F32R, name=f"xl{lb}", tag="xl")
                    nc.sync.dma_start(
                        out=xl,
                        in_=x_d[lb * 512:(lb + 1) * 512, :].rearrange(
                            "(s p) d -> p s d", p=128),
                    )
                    xt = xtpool.tile([128, NDB, 512], F32R, name=f"xt{lb}", tag="xt")
                    for db in range(NDB):
                        tp = tpsum.tile([128, 512], F32R, name=f"tp{lb}_{db}", tag="tp")
                        for s in range(4):
                            nc.tensor.transpose(
                                tp[:, s * 128:(s + 1) * 128],
                                xl[:, s, db * 128:(db + 1) * 128],
                                idf32r,
                            )
                        nc.vector.tensor_copy(xt[:, db, :], tp)

                    # Q^T / K^T blocks: out [128 head-dims, 512 l]
                    for w_sb, b_sb, dst in ((wq, bq, QT), (wk, bk, KT)):
                        for np_ in range(NPAIR):
                            qp = qkvpsum.tile([128, 512], F32,
                                              name=f"qp{lb}_{np_}", tag="qp")
                            for db in range(NDB):
                                nc.tensor.matmul(
                                    qp,
                                    w_sb[:, db, np_ * 128:(np_ + 1) * 128],
                                    xt[:, db, :],
                                    start=(db == 0), stop=False)
                            nc.tensor.matmul(
                                qp, b_sb[0:1, np_ * 128:(np_ + 1) * 128],
                                ones_r, start=False, stop=True)
                            nc.vector.tensor_copy(
                                dst[:, np_, lb * 512:(lb + 1) * 512], qp)

                    # V blocks: out [128 l, 512 head-dims]
                    for s in range(4):
                        kb = lb * 4 + s
                        vp = qkvpsum.tile([128, 512], F32, name=f"vp{kb}", tag="qp")
                        for db in range(NDB):
                            nc.tensor.matmul(
                                vp,
                                xt[:, db, s * 128:(s + 1) * 128],
                                wv[:, db, :],
                                start=(db == 0), stop=False)
                        nc.tensor.matmul(vp, ones_r[0:1, 0:128], bv,
                                         start=False, stop=True)
                        nc.vector.tensor_copy(
                            V[:, kb, :, 0:DH],
                            vp.rearrange("p (h d) -> p h d", h=HPC))

            # ================= phase 3: attention =============================
            with tc.tile_pool(name="met", bufs=1) as metp, \
                 tc.tile_pool(name="stage", bufs=2) as stagep, \
                 tc.tile_pool(name="epool", bufs=2) as epool, \
                 tc.tile_pool(name="ppool", bufs=3) as ppool, \
                 tc.tile_pool(name="oevac", bufs=2) as oevacp, \
                 tc.tile_pool(name="rpool", bufs=4) as rpool, \
                 tc.tile_pool(name="spsum", bufs=1, space="PSUM") as spsum, \
                 tc.tile_pool(name="opsum", bufs=2, space="PSUM") as opsum:

                for sb in range(NSB):
                    met = metp.tile([128, NKB, SBW], BF16,
                                    name=f"met{sb}", tag="met")
                    stage = stagep.tile([128, 8, 512], F32,
                                        name=f"st{sb}", tag="st")

                    for pr in range(NPAIR):
                        hA, hB = 2 * pr, 2 * pr + 1
                        oa = opsum.tile([DH + 1, SBW], F32,
                                        name=f"oa{sb}_{pr}", tag="o")
                        ob = opsum.tile([DH + 1, SBW], F32,
                                        name=f"ob{sb}_{pr}", tag="o")
                        for kb in range(NKB):
                            if pr == 0:
                                # stream the transposed-mask tile in just
                                # ahead of its first consumer
                                nc.sync.dma_start_transpose(
                                    met[:, kb, :],
                                    me_dram[sb * SBW:(sb + 1) * SBW,
                                            kb * 128:(kb + 1) * 128])
                            sp = spsum.tile([128, 2048], F32,
                                            name=f"sp{sb}_{pr}_{kb}", tag="s")
                            for half, rs, tpos in ((0, slice(0, 64), (0, 0)),
                                                   (1, slice(64, 128), (64, 0))):
                                for qh in range(2):
                                    nc.tensor.matmul(
                                        sp[:, half * 1024 + qh * 512:
                                           half * 1024 + qh * 512 + 512],
                                        KT[rs, pr, kb * 128:(kb + 1) * 128],
                                        QT[rs, pr, sb * SBW + qh * 512:
                                           sb * SBW + qh * 512 + 512],
                                        start=True, stop=True,
                                        tile_position=tpos)
                            ep = epool.tile([128, 2048], BF16,
                                            name=f"e{sb}_{pr}_{kb}", tag="e")
                            nc.scalar.activation(ep, sp, AF.Exp)
                            pp = ppool.tile([128, 2048], BF16,
                                            name=f"pp{sb}_{pr}_{kb}", tag="pp")
                            mdup = bass.AP(
                                tensor=met.tensor,
                                offset=met[:, kb, :].offset,
                                ap=[met.ap[0], [0, 2], [1, SBW]])
                            nc.vector.tensor_tensor(
                                out=pp.rearrange("p (r f) -> p r f", r=2),
                                in0=ep.rearrange("p (r f) -> p r f", r=2),
                                in1=mdup, op=ALU.mult)
                            for half, o_ps, h in ((0, oa, hA), (1, ob, hB)):
                                for qh in range(2):
                                    nc.tensor.matmul(
                                        o_ps[:, qh * 512:(qh + 1) * 512],
                                        V[:, kb, h, :],
                                        pp[:, half * 1024 + qh * 512:
                                           half * 1024 + qh * 512 + 512],
                                        start=(kb == 0), stop=(kb == NKB - 1))

                        # ---- postproc both heads of the pair
                        osbA = oevacp.tile([DH + 1, SBW], F32,
                                           name=f"oeA{sb}_{pr}", tag="oe")
                        osbB = oevacp.tile([DH + 1, SBW], F32,
                                           name=f"oeB{sb}_{pr}", tag="oe")
                        nc.vector.tensor_copy(osbA, oa)
                        nc.vector.tensor_copy(osbB, ob)
                        for osb, h in ((osbA, hA), (osbB, hB)):
                            hcol = h * DH
                            for g in range(2):
                                tp = opsum.tile([128, 4 * 65], F32,
                                                name=f"tp{sb}_{pr}_{h}_{g}",
                                                tag="o")
                                for j in range(4):
                                    qb = g * 4 + j
                                    nc.tensor.transpose(
                                        tp[:, j * 65:(j + 1) * 65],
                                        osb[:, qb * 128:(qb + 1) * 128],
                                        idf32[0:65, 0:65])
                                tpv = tp.rearrange("p (j c) -> p j c", j=4)
                                rec = rpool.tile([128, 4], F32,
                                                 name=f"rc{sb}_{pr}_{h}_{g}",
                                                 tag="rc")
                                nc.vector.reciprocal(rec, tpv[:, :, 64:65])
                                for j in range(4):
                                    nc.vector.tensor_scalar_mul(
                                        stage[:, g * 4 + j, hcol:hcol + DH],
                                        tpv[:, j, 0:DH],
                                        rec[:, j:j + 1])

                    for qb in range(8):
                        nc.sync.dma_start(
                            out=out_d[sb * SBW + qb * 128:
                                      sb * SBW + (qb + 1) * 128, :],
                            in_=stage[:, qb, :])

    nc.finalize()
    return nc


def _get_nc():
    if "nc" not in _CACHE:
        _CACHE["nc"] = _build()
    return _CACHE["nc"]


def kernel(embedding, mask, Wq, bq, Wk, bk, Wv, bv):
    from concourse.bass_utils import run_bass_kernel_spmd

    nc = _get_nc()

    embedding = np.asarray(embedding, dtype=np.float32)
    mask = np.asarray(mask, dtype=np.float32)
    in_maps = []
    for c in range(NCORES):
        b = c // 2
        h0 = (c % 2) * HPC
        cs = slice(h0 * DH, (h0 + HPC) * DH)
        in_maps.append({
            "x": np.ascontiguousarray(embedding[b]),
            "mask": np.ascontiguousarray(mask[b, 0]),
            "wq": np.ascontiguousarray(np.asarray(Wq, np.float32)[:, cs]),
            "wk": np.ascontiguousarray(np.asarray(Wk, np.float32)[:, cs]),
            "wv": np.ascontiguousarray(np.asarray(Wv, np.float32)[:, cs]),
            "bq": np.ascontiguousarray(np.asarray(bq, np.float32)[cs]).reshape(1, 512),
            "bk": np.ascontiguousarray(np.asarray(bk, np.float32)[cs]).reshape(1, 512),
            "bv": np.ascontiguousarray(np.asarray(bv, np.float32)[cs]).reshape(1, 512),
        })

    res = run_bass_kernel_spmd(nc, in_maps, core_ids=list(range(NCORES)))

    out = np.empty((B, L, D), dtype=np.float32)
    for c in range(NCORES):
        b = c // 2
        h0 = (c % 2) * HPC
        out[b][:, h0 * DH:(h0 + HPC) * DH] = res.results[c]["out"]
    return out


# revision 15
# speedup vs baseline: 1.2213x; 1.2069x over previous
"""Trainium2 Bass kernel for nn_Attention_3728031613575.

Multi-head attention, B=4 L=2048 D=1024 H=16 (head dim 64), fp32 reference:
    q/k/v = split_heads(x @ W{q,k,v} + b)        [b,h,l,64]
    scores = q k^T + mask * (-1e5)
    out    = softmax(scores) @ v                 -> [b,l,1024]

Sharding (8 cores): core c handles batch b = c//2 and heads (c%2)*8..+8
(batch x head-group data parallel; QKV weights column-sharded by head).
Attention is fully local per core; no collectives.

Per-core algorithm (layouts chosen so softmax lives on the PSUM partition
dim and no probability transposes are ever needed):
  - X^T built once via PE transposes (fp32r).
  - Q^T/K^T [head-dims, l] and V [l, head-dims] projections in fp32r,
    biases folded in as rank-1 matmul terms.
  - mask preprocessed once per core:  M_e = exp(-1e5*(m - rowmin(m)))
    (the rowmin bias provides exact max-subtraction for the mask-dominated
    term; the remaining q.k part is range-safe in fp32), stored bf16,
    reloaded transposed through the DMA xbar.
  - per (head-pair, q-block 512, k-block 128):
      S^T[k,q] = K^T.T @ Q^T        (two K=64 matmuls row-tiled on the PE)
      E = exp(S^T)                  (ACT, bf16, from PSUM)
      P~ = E * M_e^T                (DVE, bf16, 2-kb-wide ops)
      O'^T[d,q] += V^T P~ with a ones-column in V producing the softmax
      denominators as row 64 of O'.
  - postproc: PE-transpose O'^T -> [q, 65], reciprocal of col 64,
    tensor_scalar normalize, DMA out.

The QKV projection is emitted in l-chunks interleaved into the first
attention pair's k-loop so the PE-heavy projection hides under the
ACT-bound attention steady state; the mask phase brackets it the same way.
"""

import os
import sys
from contextlib import ExitStack

sys.path.insert(0, "/opt/trn_rl_repo")

import numpy as np

# debug knobs: phase subset + attention part subset (timeline bisection)
_PHASES = set(os.environ.get("K_PHASES", "mask,qkv,attn").split(","))
_ATTN = set(os.environ.get("K_ATTN", "qk,exp,tt,pv,post").split(","))

B, L, D, H, DH = 4, 2048, 1024, 16, 64
NCORES = 8
HPC = 8            # heads per core
NPAIR = HPC // 2   # head pairs per core
QBW = 512          # q block width
NQB = L // QBW     # 4 q blocks
NKB = L // 128     # 16 k blocks
NDB = D // 128     # 8 contraction chunks
MASK_C = -100000.0

_CACHE = {}


def _build():
    import concourse.bass as bass
    from concourse import bacc, mybir
    import concourse.tile as tile
    from concourse.masks import make_identity

    F32 = mybir.dt.float32
    F32R = mybir.dt.float32r
    BF16 = mybir.dt.bfloat16
    AF = mybir.ActivationFunctionType
    ALU = mybir.AluOpType
    AX = mybir.AxisListType

    nc = bacc.Bacc(None, target_bir_lowering=False)

    x_d = nc.dram_tensor("x", [L, D], F32R, kind="ExternalInput")
    mask_d = nc.dram_tensor("mask", [L, L], F32, kind="ExternalInput")
    wq_d = nc.dram_tensor("wq", [D, 512], F32R, kind="ExternalInput")
    wk_d = nc.dram_tensor("wk", [D, 512], F32R, kind="ExternalInput")
    wv_d = nc.dram_tensor("wv", [D, 512], F32R, kind="ExternalInput")
    bq_d = nc.dram_tensor("bq", [1, 512], F32R, kind="ExternalInput")
    bk_d = nc.dram_tensor("bk", [1, 512], F32R, kind="ExternalInput")
    bv_d = nc.dram_tensor("bv", [1, 512], F32R, kind="ExternalInput")
    out_d = nc.dram_tensor("out", [L, 512], F32, kind="ExternalOutput")

    with tile.TileContext(nc) as tc:
        with tc.tile_pool(name="const", bufs=1) as constp, \
             tc.tile_pool(name="persist", bufs=1) as pers, \
             tc.tile_pool(name="dram", bufs=1, space="DRAM") as dramp, \
             tc.tile_pool(name="met", bufs=2) as metp, \
             tc.tile_pool(name="stage", bufs=1) as stagep, \
             tc.tile_pool(name="epool", bufs=2) as epool, \
             tc.tile_pool(name="oevac", bufs=2) as oevacp, \
             tc.tile_pool(name="rpool", bufs=4) as rpool, \
             tc.tile_pool(name="spsum", bufs=2, space="PSUM") as spsum, \
             tc.tile_pool(name="opsum", bufs=2, space="PSUM") as opsum, \
             tc.tile_pool(name="scratch", bufs=2, space="PSUM") as scratch:

            # ---- constants
            idf32 = constp.tile([128, 128], F32, name="idf32", tag="idf32")
            make_identity(nc, idf32)
            idf32r = constp.tile([128, 128], F32R, name="idf32r", tag="idf32r")
            nc.vector.tensor_copy(idf32r, idf32)
            ones_f = constp.tile([1, 512], F32, name="ones_f", tag="ones_f")
            nc.vector.memset(ones_f, 1.0)
            ones_r = constp.tile([1, 512], F32R, name="ones_r", tag="ones_r")
            nc.vector.tensor_copy(ones_r, ones_f)

            # ---- persistent activations
            QT = pers.tile([128, NPAIR, L], F32R, name="QT", tag="QT")
            KT = pers.tile([128, NPAIR, L], F32R, name="KT", tag="KT")
            V = pers.tile([128, NKB, HPC, DH + 1], BF16, name="V", tag="V")
            nc.vector.memset(V[:, :, :, DH], 1.0)
            if "qkv" not in _PHASES:   # bisection aid: keep attn variant legal
                nc.vector.memset(QT, 0.5)
                nc.vector.memset(KT, 0.5)
                nc.vector.memset(V[:, :, :, 0:DH], 0.5)

            me_dram = dramp.tile([L, L], BF16, name="me_dram", tag="me_dram")

            # ---------------- helpers ------------------------------------
            def emit_mask_block(qb, mload, mtmp):
                ml = mload.tile([128, L], F32, name=f"ml{qb}", tag="ml")
                nc.sync.dma_start(out=ml,
                                  in_=mask_d[qb * 128:(qb + 1) * 128, :])
                mmin = mtmp.tile([128, 1], F32, name=f"mmin{qb}", tag="mmin")
                nc.vector.tensor_reduce(mmin, ml, axis=AX.X, op=ALU.min)
                mbias = mtmp.tile([128, 1], F32, name=f"mb{qb}", tag="mb")
                nc.vector.tensor_scalar_mul(mbias, mmin, -MASK_C)
                me = mtmp.tile([128, L], BF16, name=f"me{qb}", tag="me")
                nc.scalar.activation(me, ml, AF.Exp, bias=mbias, scale=MASK_C)
                nc.sync.dma_start(out=me_dram[qb * 128:(qb + 1) * 128, :],
                                  in_=me)

            def emit_qkv_chunk(lb, pools):
                """QKV projections for l in [lb*512, (lb+1)*512)."""
                wq, wk, wv, bq, bk, bv, xload, xtpool = pools
                xt = xtpool.tile([128, NDB, 512], F32R,
                                 name=f"xt{lb}", tag="xt")
                for sh in range(2):
                    xls = []
                    for s in range(2):
                        xl = xload.tile([128, D], F32R,
                                        name=f"xl{lb}_{sh}_{s}", tag="xl")
                        nc.sync.dma_start(
                            out=xl,
                            in_=x_d[lb * 512 + (sh * 2 + s) * 128:
                                    lb * 512 + (sh * 2 + s + 1) * 128, :])
                        xls.append(xl)
                    for db in range(NDB):
                        tpt = scratch.tile([128, 256], F32R,
                                           name=f"tpd{lb}_{sh}_{db}", tag="sc")
                        for s in range(2):
                            nc.tensor.transpose(
                                tpt[:, s * 128:(s + 1) * 128],
                                xls[s][:, db * 128:(db + 1) * 128],
                                idf32r)
                        nc.vector.tensor_copy(
                            xt[:, db, sh * 256:(sh + 1) * 256], tpt)

                for w_sb, b_sb, dst in ((wq, bq, QT), (wk, bk, KT)):
                    for np_ in range(NPAIR):
                        qp = scratch.tile([128, 512], F32,
                                          name=f"qp{lb}_{np_}_{id(w_sb) % 97}",
                                          tag="sc")
                        for db in range(NDB):
                            nc.tensor.matmul(
                                qp,
                                w_sb[:, db, np_ * 128:(np_ + 1) * 128],
                                xt[:, db, :],
                                start=(db == 0), stop=False)
                        nc.tensor.matmul(
                            qp, b_sb[0:1, np_ * 128:(np_ + 1) * 128],
                            ones_r, start=False, stop=True)
                        nc.vector.tensor_copy(
                            dst[:, np_, lb * 512:(lb + 1) * 512], qp)

                for s in range(4):
                    kb = lb * 4 + s
                    vp = scratch.tile([128, 512], F32, name=f"vp{kb}", tag="sc")
                    for db in range(NDB):
                        nc.tensor.matmul(
                            vp,
                            xt[:, db, s * 128:(s + 1) * 128],
                            wv[:, db, :],
                            start=(db == 0), stop=False)
                    nc.tensor.matmul(vp, ones_r[0:1, 0:128], bv,
                                     start=False, stop=True)
                    nc.vector.tensor_copy(
                        V[:, kb, :, 0:DH],
                        vp.rearrange("p (h d) -> p h d", h=HPC))

            met_tiles = {}

            def emit_attn_pair(qb_, pr, qkv_hook=None):
                """Attention for head pair pr on q block qb_ (512 wide)."""
                hA, hB = 2 * pr, 2 * pr + 1
                q0 = qb_ * QBW
                stage = stage_tiles[qb_]
                oa = opsum.tile([DH + 1, QBW], F32,
                                name=f"oa{qb_}_{pr}", tag="o")
                ob = opsum.tile([DH + 1, QBW], F32,
                                name=f"ob{qb_}_{pr}", tag="o")
                ep = pp = None
                for kb in range(NKB):
                    if qkv_hook is not None and kb % 4 == 0 and kb > 0:
                        qkv_hook(kb // 4)
                    if pr == 0 and kb % 8 == 0:
                        mh = metp.tile([128, 8, QBW], BF16,
                                       name=f"met{qb_}_{kb // 8}", tag="met")
                        met_tiles[(qb_, kb // 8)] = mh
                        for i in range(8):
                            nc.sync.dma_start_transpose(
                                mh[:, i, :],
                                me_dram[q0:q0 + QBW,
                                        (kb + i) * 128:(kb + i + 1) * 128])
                    sp = spsum.tile([128, 1024], F32,
                                    name=f"sp{qb_}_{pr}_{kb}", tag="s")
                    if "qk" in _ATTN:
                        nc.tensor.matmul(
                            sp[:, 0:512],
                            KT[0:64, pr, kb * 128:(kb + 1) * 128],
                            QT[0:64, pr, q0:q0 + QBW],
                            start=True, stop=True, tile_position=(0, 0))
                        nc.tensor.matmul(
                            sp[:, 512:1024],
                            KT[64:128, pr, kb * 128:(kb + 1) * 128],
                            QT[64:128, pr, q0:q0 + QBW],
                            start=True, stop=True, tile_position=(64, 0))
                    if kb % 2 == 0:
                        ep = epool.tile([128, 2, 1024], BF16,
                                        name=f"e{qb_}_{pr}_{kb}", tag="e")
                        pp = ep.rearrange("p a (b f) -> p a b f", b=2)
                    if "exp" in _ATTN:
                        nc.scalar.activation(ep[:, kb % 2, :], sp, AF.Exp)
                    if "tt" in _ATTN and kb % 2 == 1:
                        mh = met_tiles[(qb_, kb // 8)]
                        base = mh[:, (kb - 1) % 8, :]
                        mdup = bass.AP(
                            tensor=mh.tensor,
                            offset=base.offset,
                            ap=[mh.ap[0], [QBW, 2], [0, 2], [1, QBW]])
                        nc.vector.tensor_tensor(
                            out=pp,
                            in0=pp,
                            in1=mdup, op=ALU.mult)
                    if "pv" in _ATTN and kb % 2 == 1:
                        for dkb in (kb - 1, kb):
                            for o_ps, h, half in ((oa, hA, 0), (ob, hB, 1)):
                                nc.tensor.matmul(
                                    o_ps,
                                    V[:, dkb, h, :],
                                    pp[:, dkb % 2, half, :],
                                    start=(dkb == 0), stop=(dkb == NKB - 1))

                if "post" not in _ATTN or "pv" not in _ATTN:
                    return
                osbA = oevacp.tile([DH + 1, QBW], F32,
                                   name=f"oeA{qb_}_{pr}", tag="oe")
                osbB = oevacp.tile([DH + 1, QBW], F32,
                                   name=f"oeB{qb_}_{pr}", tag="oe")
                nc.vector.tensor_copy(osbA, oa)
                nc.vector.tensor_copy(osbB, ob)
                for osb, h in ((osbA, hA), (osbB, hB)):
                    hcol = h * DH
                    tp = scratch.tile([128, 4 * 65], F32,
                                      name=f"tq{qb_}_{pr}_{h}", tag="sc")
                    for j in range(4):
                        nc.tensor.transpose(
                            tp[:, j * 65:(j + 1) * 65],
                            osb[:, j * 128:(j + 1) * 128],
                            idf32[0:65, 0:65])
                    tpv = tp.rearrange("p (j c) -> p j c", j=4)
                    rec = rpool.tile([128, 4], F32,
                                     name=f"rc{qb_}_{pr}_{h}", tag="rc")
                    nc.vector.reciprocal(rec, tpv[:, :, 64:65])
                    for j in range(4):
                        nc.vector.tensor_scalar_mul(
                            stage[:, j, hcol:hcol + DH],
                            tpv[:, j, 0:DH],
                            rec[:, j:j + 1])

            def emit_out(qb_):
                stage = stage_tiles.pop(qb_)
                for j in range(4):
                    nc.sync.dma_start(
                        out=out_d[qb_ * QBW + j * 128:
                                  qb_ * QBW + (j + 1) * 128, :],
                        in_=stage[:, j, :])

            stage_tiles = {}

            # ---------------- emission schedule ---------------------------
            with tc.tile_pool(name="mloadA", bufs=2) as mload, \
                 tc.tile_pool(name="mtmpA", bufs=3) as mtmp:
                for qb in range(4 if "mask" in _PHASES else 0):
                    emit_mask_block(qb, mload, mtmp)

            qkv_stack = ExitStack()
            wpool = qkv_stack.enter_context(tc.tile_pool(name="wpool", bufs=1))
            xload = qkv_stack.enter_context(tc.tile_pool(name="xload", bufs=3))
            xtpool = qkv_stack.enter_context(tc.tile_pool(name="xtpool", bufs=1))

            if "qkv" in _PHASES:
                wq = wpool.tile([128, NDB, 512], F32R, name="wq", tag="wq")
                wk = wpool.tile([128, NDB, 512], F32R, name="wk", tag="wk")
                wv = wpool.tile([128, NDB, 512], F32R, name="wv", tag="wv")
                nc.sync.dma_start(out=wq,
                                  in_=wq_d.rearrange("(c p) n -> p c n", p=128))
                nc.sync.dma_start(out=wk,
                                  in_=wk_d.rearrange("(c p) n -> p c n", p=128))
                nc.sync.dma_start(out=wv,
                                  in_=wv_d.rearrange("(c p) n -> p c n", p=128))
                bq = wpool.tile([1, 512], F32R, name="bq", tag="bq")
                bk = wpool.tile([1, 512], F32R, name="bk", tag="bk")
                bv = wpool.tile([1, 512], F32R, name="bv", tag="bv")
                nc.sync.dma_start(out=bq, in_=bq_d[:, :])
                nc.sync.dma_start(out=bk, in_=bk_d[:, :])
                nc.sync.dma_start(out=bv, in_=bv_d[:, :])
                qkv_pools = (wq, wk, wv, bq, bk, bv, xload, xtpool)
                emit_qkv_chunk(0, qkv_pools)
                hook = (lambda lb: emit_qkv_chunk(lb, qkv_pools))
            else:
                hook = None

            if "attn" in _PHASES:
                stage_tiles[0] = stagep.tile([128, 4, 512], F32,
                                             name="st0", tag="st")
                emit_attn_pair(0, 0, qkv_hook=hook)
            elif hook is not None:
                for lb in range(1, 4):
                    hook(lb)
            qkv_stack.close()

            with tc.tile_pool(name="mloadB", bufs=2) as mload, \
                 tc.tile_pool(name="mtmpB", bufs=3) as mtmp:
                for qb in range(4, 16 if "mask" in _PHASES else 4):
                    emit_mask_block(qb, mload, mtmp)

            if "attn" in _PHASES:
                for pr in range(1, NPAIR):
                    emit_attn_pair(0, pr)
                emit_out(0)
                for qb_ in range(1, NQB):
                    stage_tiles[qb_] = stagep.tile([128, 4, 512], F32,
                                                   name=f"st{qb_}", tag="st")
                    for pr in range(NPAIR):
                        emit_attn_pair(qb_, pr)
                    emit_out(qb_)

    nc.finalize()
    return nc


def _get_nc():
    if "nc" not in _CACHE:
        _CACHE["nc"] = _build()
    return _CACHE["nc"]


def kernel(embedding, mask, Wq, bq, Wk, bk, Wv, bv):
    from concourse.bass_utils import run_bass_kernel_spmd

    nc = _get_nc()

    embedding = np.asarray(embedding, dtype=np.float32)
    mask = np.asarray(mask, dtype=np.float32)
    in_maps = []
    for c in range(NCORES):
        b = c // 2
        h0 = (c % 2) * HPC
        cs = slice(h0 * DH, (h0 + HPC) * DH)
        in_maps.append({
            "x": np.ascontiguousarray(embedding[b]),
            "mask": np.ascontiguousarray(mask[b, 0]),
            "wq": np.ascontiguousarray(np.asarray(Wq, np.float32)[:, cs]),
            "wk": np.ascontiguousarray(np.asarray(Wk, np.float32)[:, cs]),
            "wv": np.ascontiguousarray(np.asarray(Wv, np.float32)[:, cs]),
            "bq": np.ascontiguousarray(np.asarray(bq, np.float32)[cs]).reshape(1, 512),
            "bk": np.ascontiguousarray(np.asarray(bk, np.float32)[cs]).reshape(1, 512),
            "bv": np.ascontiguousarray(np.asarray(bv, np.float32)[cs]).reshape(1, 512),
        })

    res = run_bass_kernel_spmd(nc, in_maps, core_ids=list(range(NCORES)))

    out = np.empty((B, L, D), dtype=np.float32)
    for c in range(NCORES):
        b = c // 2
        h0 = (c % 2) * HPC
        out[b][:, h0 * DH:(h0 + HPC) * DH] = res.results[c]["out"]
    return out
